# revision 2
# baseline (speedup 1.0000x reference)
"""Trainium2 Bass kernel for DLRANet (4-layer low-rank MLP + log_softmax).

Strategy:
- Data-parallel over 8 NeuronCores: each core computes 1024 rows of the
  8192-row batch; the small low-rank factors K_i/Vt_i are replicated.
- Low-rank fused: never materializes W_i = K_i @ Vt_i. Per hidden layer,
  h = z @ K (contraction) and z' = relu(h @ Vt) (expansion) are computed
  chunk-by-chunk over the 4096-wide hidden dim, so the [B,4096]
  activations never hit DRAM.
- Activations are feature-major ([feature, batch]) so every matmul
  consumes K_i / Vt_i in their natural layouts; x.T is prepared host-side
  during sharding. The final layer flips back to batch-major (the h3
  chunk becomes the stationary operand), making log_softmax row-wise.
- PSUM layout: "pzp" tiles are [128, 1024] (two banks, both 512-row batch
  sub-chunks side by side), so each w-chunk needs ONE relu op at FD=1024
  instead of two at FD=512 — the ACT/DVE engines are the secondary
  bottleneck and the fixed per-op overhead (352/58 cycles) is paid half
  as often. pzp bufs=3 + hacc [128,1024] bufs=1 = 8 PSUM banks.
- Transitions 0/1 interleave both batch sub-chunks in the w-loop;
  transition 2 runs them sequentially so sub-chunk 0's final layer +
  softmax (ACT-heavy) overlaps sub-chunk 1's matmuls; final units are
  emitted spread through the w-loop (per-engine queues are FIFO, so
  emission order controls overlap).
- Matmul inputs are fp16 (1 PE cycle/row); weights/x pre-cast host-side,
  on-device activations cast by the PSUM->SBUF relu/copy.
"""

import numpy as np

_B, _DIN, _WID, _DOUT, _R = 8192, 1024, 4096, 1000, 128
_NC = 8
_BL = _B // _NC  # rows per core
_NB = 512  # batch sub-chunk (moving-operand free dim)
_NBC = _BL // _NB  # sub-chunks per core (2)
_DCH = _DIN // 128  # d-chunks in layer 0 (8)
_WCH = _WID // 128  # w-chunks per hidden layer (32)

_cache = {}


def _to_fp32r(x):
    """Round fp32 to the float32r grid (11 explicit mantissa bits, RNE)."""
    b = np.ascontiguousarray(x, np.float32).view(np.uint32).astype(np.uint64)
    rem = b & 0xFFF
    keep = b & ~np.uint64(0xFFF)
    inc = (rem > 0x800) | ((rem == 0x800) & (((b >> 12) & 1) == 1))
    out = keep + inc.astype(np.uint64) * 0x1000
    return (out & 0xFFFFFFFF).astype(np.uint32).view(np.float32)


def _chunk_major(a, p=128):
    """[C*p, F] -> [p, C*F]: partition-major layout for one contiguous DMA."""
    c = a.shape[0] // p
    return np.ascontiguousarray(
        a.reshape(c, p, a.shape[1]).transpose(1, 0, 2).reshape(p, c * a.shape[1])
    )


def build(reps=1, pin_tables=True):
    """Build + compile the per-core Bass module. reps>1 wraps the whole pass
    in a hardware For_i loop (used only for timing measurements)."""
    import os
    import concourse.bacc as bacc
    import concourse.mybir as mybir
    import concourse.tile as tile

    pin_tables = pin_tables and os.environ.get("KB_PIN", "1") == "1"
    mm_dt = os.environ.get("KB_DT", "fp16")
    # h-matmul lag (w-chunks) behind the z-matmuls in t0/t1 and t2
    lag01 = int(os.environ.get("KB_LAG01", "2"))
    lag2 = int(os.environ.get("KB_LAG2", "4"))
    # which final units to emit inside t2-bc1's w-loop, keyed by wc
    fin_wcs = [int(v) for v in os.environ.get("KB_FINWC", "7,15,23,31").split(",")]

    F32R = mybir.dt.float16 if mm_dt == "fp16" else mybir.dt.float32r
    F32 = mybir.dt.float32
    F16 = mybir.dt.float16
    AF = mybir.ActivationFunctionType

    nc = bacc.Bacc(trn_type="TRN2", target_bir_lowering=False, debug=False)

    xT_d = nc.dram_tensor("xT", [128, _DCH * _BL], F32R, kind="ExternalInput").ap()
    k_d = [
        nc.dram_tensor(
            f"k{i}",
            [128, (_DCH if i == 0 else _WCH) * _R],
            F32R,
            kind="ExternalInput",
        ).ap()
        for i in range(4)
    ]
    vt_d = [
        nc.dram_tensor(
            f"vt{i}", [128, _WID if i < 3 else _DOUT], F32R, kind="ExternalInput"
        ).ap()
        for i in range(4)
    ]
    out_d = nc.dram_tensor("out", [_BL, _DOUT], F32, kind="ExternalOutput").ap()

    with tile.TileContext(nc) as tc:
        with tc.tile_pool(name="wp", bufs=1) as wp, tc.tile_pool(
            name="hp", bufs=1
        ) as hp, tc.tile_pool(name="zp", bufs=1) as zp, tc.tile_pool(
            name="fp", bufs=1
        ) as fp, tc.tile_pool(name="ps", bufs=1, space="PSUM") as ps:

            def body():
                # ---- weight + input DMAs in need-order so compute starts as
                # soon as each piece lands ----
                NQ = 4  # quarters per 4096-wide tensor
                WQ = _WCH // NQ  # w-chunks per quarter (8)
                k0h = []
                xTh = {}

                def emit_k0_half(h):
                    kh = wp.tile([128, _DCH // 2, _R], F32R, tag=f"k0h{h}", name=f"k0h{h}")
                    nc.sync.dma_start(
                        kh[:],
                        k_d[0][
                            :, h * (_DCH // 2) * _R : (h + 1) * (_DCH // 2) * _R
                        ].rearrange("p (c r) -> p c r", c=_DCH // 2),
                    )
                    k0h.append(kh)

                def emit_xt(c):
                    for bc in range(_NBC):
                        xt = wp.tile(
                            [128, _NB], F32R, tag=f"xT{c}_{bc}", name=f"xT{c}_{bc}"
                        )
                        nc.sync.dma_start(
                            xt[:],
                            xT_d[:, c * _BL + bc * _NB : c * _BL + (bc + 1) * _NB],
                        )
                        xTh[(c, bc)] = xt

                emit_k0_half(0)
                for c in range(_DCH // 2):
                    emit_xt(c)
                emit_k0_half(1)
                for c in range(_DCH // 2, _DCH):
                    emit_xt(c)

                vt_q = [[None] * NQ for _ in range(3)]
                kn_q = [[None] * NQ for _ in range(3)]
                for i in range(3):
                    for q in range(NQ):
                        v = wp.tile(
                            [128, _WID // NQ], F32R, tag=f"vt{i}q{q}", name=f"vt{i}q{q}"
                        )
                        nc.sync.dma_start(
                            v[:],
                            vt_d[i][:, q * (_WID // NQ) : (q + 1) * (_WID // NQ)],
                        )
                        vt_q[i][q] = v
                        k = wp.tile(
                            [128, _WCH // NQ, _R],
                            F32R,
                            tag=f"k{i+1}q{q}",
                            name=f"k{i+1}q{q}",
                        )
                        nc.sync.dma_start(
                            k[:],
                            k_d[i + 1][
                                :, q * (_WID // NQ) : (q + 1) * (_WID // NQ)
                            ].rearrange("p (c r) -> p c r", c=_WCH // NQ),
                        )
                        kn_q[i][q] = k
                vt3_s = wp.tile([128, _DOUT], F32R, tag="vt3s", name="vt3s")
                nc.sync.dma_start(vt3_s[:], vt_d[3][:])

                def vt_ap(t, wc):
                    return vt_q[t][wc // WQ][:, (wc % WQ) * 128 : (wc % WQ + 1) * 128]

                def kn_ap(t, wc):
                    return kn_q[t][wc // WQ][:, wc % WQ, :]

                # ---- layer 0: h0^T[r, b] = K0^T @ x^T, accumulated over d.
                # Both batch sub-chunks accumulate into one [128,1024] psum
                # tile so the PSUM->SBUF cast is a single FD=1024 op. ----
                haccT = ps.tile([128, 2 * _NB], F32, tag="hacc", name="hacc0")
                for c in range(_DCH):
                    for bc in range(_NBC):
                        nc.tensor.matmul(
                            haccT[:, bc * _NB : (bc + 1) * _NB],
                            k0h[c // (_DCH // 2)][:, c % (_DCH // 2), :],
                            xTh[(c, bc)][:],
                            start=(c == 0),
                            stop=(c == _DCH - 1),
                        )
                h_cur = hp.tile([128, 2 * _NB], F32R, tag="h", bufs=4, name="h0")
                nc.vector.tensor_copy(h_cur[:], haccT[:])

                # ---- transitions 0,1: both batch sub-chunks interleaved in
                # the w-loop; one FD=1024 relu per w-chunk, alternating
                # ACT/DVE; h-matmuls run lag01 w-chunks behind ----
                for t in range(2):
                    haccT = ps.tile([128, 2 * _NB], F32, tag="hacc", name=f"hacc{t+1}")
                    zs_live = {}
                    for wc in range(_WCH + lag01):
                        if wc < _WCH:
                            pzp = ps.tile(
                                [128, 2 * _NB], F32, tag="pzp", bufs=3,
                                name=f"pz{t}_{wc}",
                            )
                            for bc in range(_NBC):
                                nc.tensor.matmul(
                                    pzp[:, bc * _NB : (bc + 1) * _NB],
                                    vt_ap(t, wc),
                                    h_cur[:, bc * _NB : (bc + 1) * _NB],
                                    start=True,
                                    stop=True,
                                )
                            zt = zp.tile(
                                [128, 2 * _NB], F32R, tag="zs", bufs=6,
                                name=f"zs{t}_{wc}",
                            )
                            if wc % 2 == 0:
                                nc.scalar.activation(zt[:], pzp[:], AF.Relu)
                            else:
                                nc.vector.tensor_scalar_max(zt[:], pzp[:], 0.0)
                            zs_live[wc] = zt
                        if wc >= lag01:
                            zprev = zs_live.pop(wc - lag01)
                            for bc in range(_NBC):
                                nc.tensor.matmul(
                                    haccT[:, bc * _NB : (bc + 1) * _NB],
                                    kn_ap(t, wc - lag01),
                                    zprev[:, bc * _NB : (bc + 1) * _NB],
                                    start=(wc == lag01),
                                    stop=(wc == _WCH + lag01 - 1),
                                )
                    h_nxt = hp.tile([128, 2 * _NB], F32R, tag="h", bufs=4, name=f"h{t+1}")
                    nc.vector.tensor_copy(h_nxt[:], haccT[:])
                    h_cur = h_nxt

                # ---- final layer + log_softmax for one 128-row batch chunk.
                # logits land in a pzp psum tile; out = logits - ln(sum(exp)).
                # Logits are O(1) so exp without max-subtraction is safe. ----
                def emit_final_chunk(g, h3_tile, j):
                    lhsT = h3_tile[:, j * 128 : (j + 1) * 128]
                    lgp = ps.tile([128, 2 * _NB], F32, tag="pzp", bufs=3, name=f"lgp{g}")
                    nc.tensor.matmul(
                        lgp[:, 0:_NB], lhsT, vt3_s[:, 0:_NB], start=True, stop=True
                    )
                    nc.tensor.matmul(
                        lgp[:, _NB:_DOUT],
                        lhsT,
                        vt3_s[:, _NB:_DOUT],
                        start=True,
                        stop=True,
                    )
                    lg = lgp[:, 0:_DOUT]
                    e_s = fp.tile([128, _DOUT], F16, tag="e", bufs=2, name=f"e{g}")
                    ssum = fp.tile([128, 1], F32, tag="ss", bufs=2, name=f"ss{g}")
                    nc.scalar.activation(e_s[:], lg[:], AF.Exp, accum_out=ssum[:])
                    lns = fp.tile([128, 1], F32, tag="lns", bufs=2, name=f"lns{g}")
                    nc.scalar.activation(lns[:], ssum[:], AF.Ln)
                    o_s = fp.tile([128, _DOUT], F32, tag="os", bufs=3, name=f"os{g}")
                    nc.vector.tensor_scalar_sub(o_s[:], lg[:], lns[:])
                    nc.sync.dma_start(out_d[g * 128 : (g + 1) * 128, :], o_s[:])

                # ---- transition 2: one batch sub-chunk at a time; w-chunk
                # pairs share a [128,1024] psum tile -> one FD=1024 relu per
                # pair; sub-chunk 0's final units are emitted spread through
                # sub-chunk 1's w-loop ----
                haccT2 = ps.tile([128, 2 * _NB], F32, tag="hacc", name="hacc3")
                h3 = [None, None]
                for bc in range(_NBC):
                    zs_live = {}
                    pzp = None
                    for wc in range(_WCH + lag2):
                        if wc < _WCH:
                            if wc % 2 == 0:
                                pzp = ps.tile(
                                    [128, 2 * _NB], F32, tag="pzp", bufs=3,
                                    name=f"pzt2_{bc}_{wc}",
                                )
                            nc.tensor.matmul(
                                pzp[:, (wc % 2) * _NB : (wc % 2 + 1) * _NB],
                                vt_ap(2, wc),
                                h_cur[:, bc * _NB : (bc + 1) * _NB],
                                start=True,
                                stop=True,
                            )
                            if wc % 2 == 1:
                                zt = zp.tile(
                                    [128, 2 * _NB], F32R, tag="zs", bufs=6,
                                    name=f"zt2_{bc}_{wc}",
                                )
                                if (wc // 2) % 2 == 0:
                                    nc.scalar.activation(zt[:], pzp[:], AF.Relu)
                                else:
                                    nc.vector.tensor_scalar_max(zt[:], pzp[:], 0.0)
                                zs_live[wc - 1] = zt[:, 0:_NB]
                                zs_live[wc] = zt[:, _NB : 2 * _NB]
                        if wc >= lag2:
                            nc.tensor.matmul(
                                haccT2[:, bc * _NB : (bc + 1) * _NB],
                                kn_ap(2, wc - lag2),
                                zs_live.pop(wc - lag2),
                                start=(wc == lag2),
                                stop=(wc == _WCH + lag2 - 1),
                            )
                        if bc == 1 and wc in fin_wcs:
                            g = fin_wcs.index(wc)
                            emit_final_chunk(g, h3[0], g)
                    h3[bc] = hp.tile(
                        [128, _NB], F32R, tag="h3", bufs=2, name=f"h3_{bc}"
                    )
                    nc.vector.tensor_copy(
                        h3[bc][:], haccT2[:, bc * _NB : (bc + 1) * _NB]
                    )
                for j in range(_NB // 128):
                    emit_final_chunk(4 + j, h3[1], j)

            if reps == 1:
                body()
            else:
                with tc.For_i(0, reps):
                    body()

    # All activation funcs used here (Relu/Copy/Identity/Exp/Ln) coexist in
    # act-func-set "natural_log_exp_and_others". Left alone, the table-load
    # pass picks the first set containing each func (exp->set0, ln->set5),
    # thrashing ~1.3us table loads between them. Restrict every other set's
    # advertised funcs so all activations resolve to that one set -> a single
    # table load for the whole kernel.
    import concourse.bacc as bacc_mod
    from concourse.hw_specs import get_activation_tables as _real_tables

    if not pin_tables:
        nc.compile()
        return nc

    def _pinned_tables(arch):
        tabs = _real_tables(arch)
        pinned = "natural_log_exp_and_others"
        if pinned in tabs:
            ours = tabs[pinned]
            tabs = {
                name: (funcs if name == pinned else (funcs - ours))
                for name, funcs in tabs.items()
            }
        return tabs

    bacc_mod.get_activation_tables = _pinned_tables
    try:
        nc.compile()
    finally:
        bacc_mod.get_activation_tables = _real_tables
    return nc


def _prep_inputs(x, K0, Vt0, K1, Vt1, K2, Vt2, K3, Vt3):
    """Host-side sharding + layout prep: cast to the matmul dtype (fp16 by
    default), chunk-major weights, per-core transposed x shards."""
    import os

    if os.environ.get("KB_DT", "fp16") == "fp16":
        cast = lambda a: np.asarray(a, np.float32).astype(np.float16)
    else:
        cast = lambda a: _to_fp32r(np.ascontiguousarray(a, np.float32))
    ks = [_chunk_major(cast(np.asarray(k, np.float32))) for k in (K0, K1, K2, K3)]
    vts = [cast(np.ascontiguousarray(v, np.float32)) for v in (Vt0, Vt1, Vt2, Vt3)]
    xr = cast(np.asarray(x, np.float32))
    in_maps = []
    for c in range(_NC):
        xT = _chunk_major(np.ascontiguousarray(xr[c * _BL : (c + 1) * _BL].T))
        m = {"xT": xT}
        for i in range(4):
            m[f"k{i}"] = ks[i]
            m[f"vt{i}"] = vts[i]
        in_maps.append(m)
    return in_maps


def kernel(x, K0, Vt0, K1, Vt1, K2, Vt2, K3, Vt3):
    from concourse import bass_utils

    if "nc" not in _cache:
        _cache["nc"] = build(reps=1)
    nc = _cache["nc"]
    in_maps = _prep_inputs(x, K0, Vt0, K1, Vt1, K2, Vt2, K3, Vt3)
    res = bass_utils.run_bass_kernel_spmd(nc, in_maps, core_ids=list(range(_NC)))
    return np.concatenate([r["out"] for r in res.results], axis=0)
